# revision 1
# baseline (speedup 1.0000x reference)
"""Trainium2 Bass kernel for nn_DRL4SSP (pointer-network greedy decode).

Strategy: pure data-parallel over batch B=64 across 8 NeuronCores (8 items
per core). Inside each core the 127 sequential decode steps run fully
on-chip: encoders/bases are computed once in a prologue; the per-step
recurrence (GRU + two pointer-attention stages + greedy argmax) is executed
with all state resident in SBUF/PSUM. Two pipeline groups of 4 batch items
interleave to hide the cross-engine dependency chain.

Key layout choices (per core, b = local batch 0..7, s = position 0..127):
  base1P/base2P   [128(h), 1024(b-major, s)]   loop-invariant bias tensors
  W2SHT/WdecST    [128(s), 1024(b-major, h)]   per-item folded weights
  attn/softmax    [128(s), nb] transposed form; softmax sums are
                  partition-replicated via a ones-matrix matmul so stage 1
                  needs no partition reshapes at all.
  logits          transposed->block via one PE transpose; argmax via DVE
                  max/max_index; log-prob denominators deferred to a single
                  post-loop pass (keeps the hot loop on one ACT table set).
All compute is fp32: bf16 was measured to flip 63/64 tours and float32r
(TF32-class, ~2e-4 rounding) flipped 18/64, so the broadcast-adds run as
DVE tensor-tensor adds with 0-stride APs instead of PE identity-matmuls.
"""
import sys
import numpy as np

for _p in ("/opt/trn_rl_repo",):
    if _p not in sys.path:
        sys.path.insert(0, _p)

B, SS, DS, H, S = 64, 8, 4, 128, 128
NCORES = 8
BL = B // NCORES          # batch items per core = 8
NG = 2                    # pipeline groups per core
GB = BL // NG             # batch items per group = 4
NSTEP = S - 1             # 127
NEG = -1e30


def _build_nc(n_steps=NSTEP, bench_loop=1):
    from contextlib import ExitStack
    import concourse.bass as bass
    import concourse.tile as tile
    from concourse import bacc, mybir

    f32 = mybir.dt.float32
    f32r = mybir.dt.float32r
    u32 = mybir.dt.uint32
    AF = mybir.ActivationFunctionType
    OP = mybir.AluOpType

    nc = bacc.Bacc("TRN2", target_bir_lowering=False, debug=False,
                   enable_asserts=False)

    # ---- DRAM I/O ----
    din = {}
    def dram_in(name, shape):
        din[name] = nc.dram_tensor(name, shape, f32, kind="ExternalInput").ap()
    dram_in("staticT8", [SS, BL * S])      # [i, (b,s)]
    dram_in("dynT4", [DS, BL * S])
    dram_in("penT0", [S, BL])              # penalty, transposed [s, b]
    for nm, shp in [("WsT", [SS, H]), ("WdT", [DS, H]),
                    ("ww1sT", [H, H]), ("ww1dT", [H, H]), ("w1hT", [H, H]),
                    ("ww2sT", [H, H]), ("ww2dT", [H, H]), ("w2dT", [H, H]),
                    ("WdecT", [SS, H]),
                    ("WihT_r", [H, H]), ("WihT_z", [H, H]), ("WihT_n", [H, H]),
                    ("WhhT_r", [H, H]), ("WhhT_z", [H, H]), ("WhhT_nh", [H, H]),
                    ("vv1c", [H, 1]), ("vv2c", [H, 1]),
                    ("I128", [H, H]), ("ones128", [H, H])]:
        dram_in(nm, shp)
    nchunk_o = (GB * n_steps + S - 1) // S
    out_idx = nc.dram_tensor("out_idx_raw", [BL, n_steps], u32,
                             kind="ExternalOutput").ap()
    out_logp = nc.dram_tensor("out_logp_raw", [H, NG * nchunk_o], f32,
                              kind="ExternalOutput").ap()

    r = lambda ap: ap.bitcast(f32r)

    with ExitStack() as ctx:
        tc = ctx.enter_context(tile.TileContext(nc))
        cpool = ctx.enter_context(tc.tile_pool(name="consts", bufs=1))
        state = ctx.enter_context(tc.tile_pool(name="state", bufs=1))
        work = ctx.enter_context(tc.tile_pool(name="work", bufs=3))

        if bench_loop > 1:
            loop_cm = tc.For_i(0, bench_loop, 1)
        else:
            loop_cm = None
        from contextlib import nullcontext
        with (loop_cm if loop_cm is not None else nullcontext()):
            # ---- load constants to SBUF ----
            # Every const is copied once through DVE so that downstream matmuls
            # depend on a single engine semaphore (fp32 self-loading matmuls
            # tolerate only one sync wait).
            cs = {}
            for nm, ap in din.items():
                raw = cpool.tile(list(ap.shape), f32, tag=f"r_{nm}", name=f"r_{nm}")
                nc.sync.dma_start(raw[:], ap[:])
                t = cpool.tile(list(ap.shape), f32, tag=nm, name=f"c_{nm}")
                nc.vector.tensor_copy(out=t[:], in_=raw[:])
                cs[nm] = t

            # ---- persistent state ----
            base1P = state.tile([H, BL * S], f32, tag="base1P")
            base2P = state.tile([H, BL * S], f32, tag="base2P")
            W2SHT = state.tile([S, BL * H], f32, tag="W2SHT")
            WdecST = state.tile([S, BL * H], f32, tag="WdecST")
            hT = [state.tile([H, GB], f32, tag=f"hT{g}", name=f"hT_{g}")
                  for g in range(NG)]
            dec_hT = [state.tile([H, GB], f32, tag=f"dhT{g}", name=f"dhT_{g}")
                      for g in range(NG)]
            penaltyT = [state.tile([S, GB], f32, tag=f"penT{g}", name=f"penT_{g}")
                        for g in range(NG)]
            # per-group logit store in transposed [s, (t,b)] form + ptr store
            logbT = [state.tile([S, GB * n_steps], f32, tag=f"logbT{g}",
                                name=f"logbT_{g}") for g in range(NG)]
            ptrb = [state.tile([GB, n_steps], u32, tag=f"ptrb{g}",
                               name=f"ptrb_{g}") for g in range(NG)]
            shS = state.tile([H, BL * S], f32, tag="shS")       # static_h
            dhS = state.tile([H, BL * S], f32, tag="dhS")       # dynamic_h

            for g in range(NG):
                nc.vector.memset(hT[g][:], 0.0)
                nc.vector.memset(dec_hT[g][:], 0.0)
                nc.vector.memset(logbT[g][:], 0.0)
                nc.vector.tensor_copy(out=penaltyT[g][:],
                                      in_=cs["penT0"][:, g * GB:(g + 1) * GB])

            # ---- prologue: encoders, bases, folded weights ----
            with tc.tile_pool(name="pro_ps", bufs=2, space="PSUM") as pps:
                def big_mm_to(dst, terms):
                    # dst[:, h0:h0+512] accumulated from [(lhsT, rhs)] fp32r mms
                    for half in range(2):
                        sl = slice(half * 512, half * 512 + 512)
                        pt = pps.tile([H, 512], f32, tag="pro")
                        for i, (lhsT, rhs) in enumerate(terms):
                            nc.tensor.matmul(pt[:], lhsT, rhs[:, sl],
                                             start=(i == 0),
                                             stop=(i == len(terms) - 1))
                        nc.vector.tensor_copy(out=dst[:, sl], in_=pt[:])

                big_mm_to(shS, [(cs["WsT"][:], cs["staticT8"][:])])
                big_mm_to(dhS, [(cs["WdT"][:], cs["dynT4"][:])])
                big_mm_to(base1P, [(cs["ww1sT"][:], shS[:]),
                                   (cs["ww1dT"][:], dhS[:])])
                big_mm_to(base2P, [(cs["ww2sT"][:], shS[:]),
                                   (cs["ww2dT"][:], dhS[:])])

                # W2SH = w2d @ static_h, then per-item transpose to [s, (b,h)]
                w2a = state.tile([H, BL * S], f32, tag="w2a")
                big_mm_to(w2a, [(cs["w2dT"][:], shS[:])])
                wda = state.tile([H, BL * S], f32, tag="wda")
                big_mm_to(wda, [(cs["WdecT"][:], cs["staticT8"][:])])
                for b in range(BL):
                    sl = slice(b * S, b * S + S)
                    pt = pps.tile([H, S], f32, tag="protr")
                    nc.tensor.transpose(pt[:], w2a[:, sl], cs["I128"][:])
                    nc.vector.tensor_copy(out=W2SHT[:, sl], in_=pt[:])
                    pt2 = pps.tile([H, S], f32, tag="protr")
                    nc.tensor.transpose(pt2[:], wda[:, sl], cs["I128"][:])
                    nc.vector.tensor_copy(out=WdecST[:, sl], in_=pt2[:])

            # ---- main-loop PSUM pools (per group) ----
            psA = [ctx.enter_context(
                tc.tile_pool(name=f"Ag{g}", bufs=1, space="PSUM")) for g in range(NG)]
            psB = [ctx.enter_context(
                tc.tile_pool(name=f"Bg{g}", bufs=1, space="PSUM")) for g in range(NG)]

            # bankA: gates r|z (0:8), nacc (8:12), hn2 (12:16), A1T (16:20),
            #        S1rep (20:24), DH (24:28), U1 (28:32)
            bkA = [psA[g].tile([H, 512], f32, tag="bka", name=f"bkA_{g}") for g in range(NG)]
            # bankB: U2 (0:4), A2T (4:8), OHT (8:12), Lblk [0:4, 16:144]
            bkB = [psB[g].tile([H, 512], f32, tag="bkb", name=f"bkB_{g}") for g in range(NG)]

            AFt, AFe = AF.Tanh, AF.Exp

            def step(t, g):
                gc = slice(g * GB, g * GB + GB)          # group batch cols
                gs = slice(g * GB * S, (g + 1) * GB * S)  # group (b,s) cols
                gh = slice(g * GB * H, (g + 1) * GB * H)  # group (b,h) cols
                ga, gb_ = bkA[g], bkB[g]
                G_r, G_z = ga[:, 0:4], ga[:, 4:8]
                G_rz, G_n, G_h2 = ga[:, 0:8], ga[:, 8:12], ga[:, 12:16]
                A1T, S1rep, DH = ga[:, 16:20], ga[:, 20:24], ga[:, 24:28]
                U1 = ga[:, 28:32]
                U2, A2T, OHT = gb_[:, 0:4], gb_[:, 4:8], gb_[:, 8:12]
                Lblk = gb_[0:GB, 16:144]
                dh_g, h_g = dec_hT[g][:], hT[g][:]

                # ---- GRU ----
                nc.tensor.matmul(G_h2, cs["WhhT_nh"][:], h_g, start=True, stop=True)
                nc.tensor.matmul(G_r, cs["WihT_r"][:], dh_g, start=True, stop=False)
                nc.tensor.matmul(G_r, cs["WhhT_r"][:], h_g, start=False, stop=True)
                nc.tensor.matmul(G_z, cs["WihT_z"][:], dh_g, start=True, stop=False)
                nc.tensor.matmul(G_z, cs["WhhT_z"][:], h_g, start=False, stop=True)
                nc.tensor.matmul(G_n, cs["WihT_n"][:], dh_g, start=True, stop=True)
                trz = work.tile([H, 2 * GB], f32, tag=f"trz{g}")
                nc.scalar.activation(trz[:], G_rz, AFt, scale=0.5)
                q2 = work.tile([H, GB], f32, tag=f"q2{g}")
                nc.vector.tensor_scalar(out=q2[:], in0=trz[:, 0:GB],
                                        scalar1=1.0, scalar2=None, op0=OP.add)
                q = work.tile([H, GB], f32, tag=f"q{g}")
                nc.vector.tensor_tensor(out=q[:], in0=q2[:], in1=G_h2,
                                        op=OP.mult)
                nin = work.tile([H, GB], f32, tag=f"nin{g}")
                nc.vector.tensor_tensor(out=nin[:], in0=q[:], in1=G_n, op=OP.add)
                tn = work.tile([H, GB], f32, tag=f"tn{g}")
                nc.scalar.activation(tn[:], nin[:], AFt)
                z2 = work.tile([H, GB], f32, tag=f"z2{g}")
                nc.vector.tensor_scalar(out=z2[:], in0=trz[:, GB:2 * GB],
                                        scalar1=0.5, scalar2=0.5,
                                        op0=OP.mult, op1=OP.add)
                v = work.tile([H, GB], f32, tag=f"v{g}")
                nc.vector.tensor_tensor(out=v[:], in0=h_g, in1=tn[:],
                                        op=OP.subtract)
                w_ = work.tile([H, GB], f32, tag=f"w{g}")
                nc.vector.tensor_tensor(out=w_[:], in0=z2[:], in1=v[:], op=OP.mult)
                nc.vector.tensor_tensor(out=h_g, in0=tn[:], in1=w_[:], op=OP.add)

                # ---- stage 1: t1 = tanh(base1 + u1), u1 = w1h @ h ----
                nc.tensor.matmul(U1, cs["w1hT"][:], h_g, start=True, stop=True)
                t1pre = work.tile([H, GB * S], f32, tag=f"t1p{g}")
                for cb in range(2):
                    cw = slice(cb * 256, cb * 256 + 256)
                    gsc = slice(g * GB * S + cb * 256, g * GB * S + cb * 256 + 256)
                    nc.vector.tensor_tensor(
                        out=t1pre[:, cw].rearrange("p (b s) -> p b s", b=2),
                        in0=base1P[:, gsc].rearrange("p (b s) -> p b s", b=2),
                        in1=U1[:, 2 * cb:2 * cb + 2, None]
                            .broadcast_to((H, 2, S)), op=OP.add)
                t1S = work.tile([H, GB * S], f32, tag=f"t1S{g}")
                nc.scalar.activation(t1S[:, 0:256], t1pre[:, 0:256], AFt)
                nc.scalar.activation(t1S[:, 256:512], t1pre[:, 256:512], AFt)
                for bl in range(GB):
                    nc.tensor.matmul(A1T[:, bl:bl + 1],
                                     t1S[:, bl * S:(bl + 1) * S], cs["vv1c"][:],
                                     start=True, stop=True)
                e1T = work.tile([S, GB], f32, tag=f"e1T{g}")
                nc.scalar.activation(e1T[:], A1T, AFe)   # softmax1 w/o max-sub
                nc.tensor.matmul(S1rep, cs["ones128"][:], e1T[:],
                                 start=True, stop=True)
                r1 = work.tile([S, GB], f32, tag=f"r1{g}")
                nc.vector.reciprocal(r1[:], S1rep)
                e1sT = work.tile([S, GB], f32, tag=f"e1sT{g}")
                nc.vector.tensor_tensor(out=e1sT[:], in0=e1T[:], in1=r1[:],
                                        op=OP.mult)

                # ---- stage 2: t2 = tanh(base2 + u2), u2 = W2SH @ softmax1 ----
                for bl in range(GB):
                    b = g * GB + bl
                    nc.tensor.matmul(U2[:, bl:bl + 1],
                                     W2SHT[:, b * H:(b + 1) * H],
                                     e1sT[:, bl:bl + 1], start=True, stop=True)
                u2S = work.tile([H, GB], f32, tag=f"u2S{g}")
                nc.vector.tensor_copy(out=u2S[:], in_=U2)
                t2pre = work.tile([H, GB * S], f32, tag=f"t2p{g}")
                for cb in range(2):
                    cw = slice(cb * 256, cb * 256 + 256)
                    gsc = slice(g * GB * S + cb * 256, g * GB * S + cb * 256 + 256)
                    nc.vector.tensor_tensor(
                        out=t2pre[:, cw].rearrange("p (b s) -> p b s", b=2),
                        in0=base2P[:, gsc].rearrange("p (b s) -> p b s", b=2),
                        in1=u2S[:, 2 * cb:2 * cb + 2, None]
                            .broadcast_to((H, 2, S)), op=OP.add)
                t2S = work.tile([H, GB * S], f32, tag=f"t2S{g}")
                nc.scalar.activation(t2S[:, 0:256], t2pre[:, 0:256], AFt)
                nc.scalar.activation(t2S[:, 256:512], t2pre[:, 256:512], AFt)
                for bl in range(GB):
                    nc.tensor.matmul(A2T[:, bl:bl + 1],
                                     t2S[:, bl * S:(bl + 1) * S], cs["vv2c"][:],
                                     start=True, stop=True)

                # ---- logits, argmax, bookkeeping ----
                logitsT = work.tile([S, GB], f32, tag=f"lgT{g}")
                nc.vector.tensor_tensor(out=logitsT[:], in0=A2T,
                                        in1=penaltyT[g][:], op=OP.add)
                nc.vector.tensor_copy(out=logbT[g][:, t * GB:(t + 1) * GB],
                                      in_=logitsT[:])
                nc.tensor.transpose(Lblk, logitsT[:], cs["I128"][:])
                LS = work.tile([GB, S], f32, tag=f"ls{g}")
                nc.vector.tensor_copy(out=LS[:], in_=Lblk)
                M8 = work.tile([GB, 8], f32, tag=f"m8{g}")
                nc.vector.max(M8[:], LS[:])
                I8u = work.tile([GB, 8], u32, tag=f"i8{g}")
                nc.vector.max_index(I8u[:], M8[:], LS[:])
                nc.vector.tensor_copy(out=ptrb[g][:, t:t + 1], in_=I8u[:, 0:1])
                OHb = work.tile([GB, S], f32, tag=f"oh{g}")
                nc.vector.tensor_scalar(out=OHb[:], in0=LS[:], scalar1=M8[:, 0:1],
                                        scalar2=None, op0=OP.is_equal)
                nc.tensor.transpose(OHT, OHb[:], cs["I128"][0:GB, 0:GB])
                ohT = work.tile([S, GB], f32, tag=f"ohT{g}")
                nc.vector.tensor_copy(out=ohT[:], in_=OHT)
                # next-step decoder input: dec_h = (W_dec @ static)[:, :, ptr]
                for bl in range(GB):
                    b = g * GB + bl
                    nc.tensor.matmul(DH[:, bl:bl + 1],
                                     WdecST[:, b * H:(b + 1) * H],
                                     ohT[:, bl:bl + 1], start=True, stop=True)
                nc.vector.tensor_copy(out=dec_hT[g][:], in_=DH)
                # penalty update (gpsimd, off critical path)
                tsp = work.tile([S, GB], f32, tag=f"tsp{g}")
                nc.gpsimd.tensor_scalar(out=tsp[:], in0=ohT[:], scalar1=NEG,
                                        scalar2=None, op0=OP.mult)
                nc.gpsimd.tensor_tensor(out=penaltyT[g][:], in0=penaltyT[g][:],
                                        in1=tsp[:], op=OP.add)

            for t in range(n_steps):
                for g in range(NG):
                    step(t, g)

            # ---- post-loop: logp = -ln(sum(exp(logits - max))) ----
            # logbT[g] is [s, (t,b)]; transpose 128-col chunks to [(t,b), s],
            # then exp(bias=-max) with fused row-sum, then ln, negate.
            nchunk = (GB * n_steps + S - 1) // S          # chunks per group
            sums = [state.tile([S, nchunk], f32, tag=f"sums{g}",
                               name=f"sums_{g}") for g in range(NG)]
            for g in range(NG):
                nc.vector.memset(sums[g][:], 1.0)
                for c in range(nchunk):
                    w0 = c * S
                    wid = min(S, GB * n_steps - w0)
                    pt = psB[g].tile([S, S], f32, tag="bkb", name=f"pT{g}{c}")
                    nc.tensor.transpose(pt[0:wid, :],
                                        logbT[g][:, w0:w0 + wid], cs["I128"][:])
                    blk = work.tile([S, S], f32, tag=f"pb{g}")
                    nc.vector.tensor_copy(out=blk[0:wid, :], in_=pt[0:wid, :])
                    nmx = work.tile([S, 1], f32, tag=f"nm{g}")
                    nc.vector.tensor_reduce(out=nmx[0:wid, :], in_=blk[0:wid, :],
                                            op=OP.max,
                                            axis=mybir.AxisListType.X,
                                            negate=True)
                    eb = work.tile([S, S], f32, tag=f"eb{g}")
                    nc.scalar.activation(eb[0:wid, :], blk[0:wid, :], AFe,
                                         bias=nmx[0:wid, :],
                                         accum_out=sums[g][0:wid, c:c + 1])
            logpb = [state.tile([S, nchunk], f32, tag=f"logpb{g}",
                                name=f"logpb_{g}") for g in range(NG)]
            for g in range(NG):
                lnb = work.tile([S, nchunk], f32, tag=f"lnb{g}")
                nc.scalar.activation(lnb[:], sums[g][:], AF.Ln)
                nc.vector.tensor_scalar(out=logpb[g][:], in0=lnb[:], scalar1=-1.0,
                                        scalar2=None, op0=OP.mult)
                nc.sync.dma_start(out_idx[g * GB:(g + 1) * GB, :], ptrb[g][:])
                nc.sync.dma_start(out_logp[:, g * nchunk:(g + 1) * nchunk],
                                  logpb[g][:])

    nc.compile()
    return nc


def host_inputs(static, dynamic, W_s, W_d, W_dec, vv1, ww1, vv2, ww2,
                W_ih, W_hh):
    """Per-core in_maps (layout transforms only; all heavy compute on-device)."""
    f = np.float32
    shared = {
        "WsT": np.ascontiguousarray(W_s.T, f),
        "WdT": np.ascontiguousarray(W_d.T, f),
        "ww1sT": np.ascontiguousarray(ww1[:, :H].T, f),
        "ww1dT": np.ascontiguousarray(ww1[:, H:2 * H].T, f),
        "w1hT": np.ascontiguousarray(ww1[:, 2 * H:].T, f),
        "ww2sT": np.ascontiguousarray(ww2[:, :H].T, f),
        "ww2dT": np.ascontiguousarray(ww2[:, 2 * H:].T, f),
        "w2dT": np.ascontiguousarray(ww2[:, H:2 * H].T, f),
        "WdecT": np.ascontiguousarray(W_dec.T, f),
        "WihT_r": np.ascontiguousarray(W_ih[:H].T, f),
        "WihT_z": np.ascontiguousarray(W_ih[H:2 * H].T, f),
        "WihT_n": np.ascontiguousarray(W_ih[2 * H:].T, f),
        "WhhT_r": np.ascontiguousarray(W_hh[:H].T, f),
        "WhhT_z": np.ascontiguousarray(W_hh[H:2 * H].T, f),
        "WhhT_nh": np.ascontiguousarray(0.5 * W_hh[2 * H:].T, f),
        "vv1c": np.ascontiguousarray(vv1[:, None], f),
        "vv2c": np.ascontiguousarray(vv2[:, None], f),
        "I128": np.eye(H, dtype=f),
        "ones128": np.ones((H, H), f),
    }
    in_maps = []
    for c in range(NCORES):
        bs = slice(c * BL, (c + 1) * BL)
        pen = np.where(dynamic[bs, 0, :] != 0, NEG, 0.0).astype(f)
        pen[:, 0] = NEG
        m = dict(shared)
        m["staticT8"] = np.ascontiguousarray(
            static[bs].transpose(1, 0, 2).reshape(SS, BL * S), f)
        m["dynT4"] = np.ascontiguousarray(
            dynamic[bs].transpose(1, 0, 2).reshape(DS, BL * S), f)
        m["penT0"] = np.ascontiguousarray(pen.T, f)
        in_maps.append(m)
    return in_maps


def unpack_outputs(results, n_steps=NSTEP):
    """results: list of 8 dicts with out_idx_raw/out_logp_raw."""
    nchunk = (GB * n_steps + S - 1) // S
    idxs, logps = [], []
    for res in results:
        idxs.append(res["out_idx_raw"].astype(np.int32))
        raw = res["out_logp_raw"]
        lp = np.zeros((BL, n_steps), np.float32)
        for g in range(NG):
            flat = raw[:, g * nchunk:(g + 1) * nchunk].T.reshape(-1)
            lp[g * GB:(g + 1) * GB, :] = \
                flat[:GB * n_steps].reshape(n_steps, GB).T
        logps.append(lp)
    return np.concatenate(idxs, 0), np.concatenate(logps, 0)


_CACHE = {}


def kernel(static, dynamic, transition_time, W_s, b_s, W_d, b_d, W_dec, b_dec,
           vv1, ww1, vv2, ww2, W_ih, W_hh, b_ih, b_hh):
    for bias in (b_s, b_d, b_dec, b_ih, b_hh):
        assert not np.any(np.asarray(bias)), "kernel assumes zero biases"
    from concourse.bass_utils import run_bass_kernel_spmd
    if "nc" not in _CACHE:
        _CACHE["nc"] = _build_nc()
    in_maps = host_inputs(np.asarray(static), np.asarray(dynamic),
                          np.asarray(W_s), np.asarray(W_d), np.asarray(W_dec),
                          np.asarray(vv1), np.asarray(ww1), np.asarray(vv2),
                          np.asarray(ww2), np.asarray(W_ih), np.asarray(W_hh))
    res = run_bass_kernel_spmd(_CACHE["nc"], in_maps,
                               core_ids=list(range(NCORES)))
    return unpack_outputs(res.results)



# revision 16
# speedup vs baseline: 1.5384x; 1.5384x over previous
"""Trainium2 Bass kernel for nn_DRL4SSP (pointer-network greedy decode).

Strategy: pure data-parallel over batch B=64 across 8 NeuronCores (8 items
per core), two pipeline groups of 4 items interleaved per core. The 127
decode steps are latency-bound on the cross-engine dependency chain, so the
step is built to minimise serial stages:

  - GRU input path folded: M_x = W_ih_x @ W_dec is folded on the host, and
    (M_x @ static_b).T slabs are built in the prologue directly as
    static_b.T @ M_x.T (no transpose pass), so the one-hot from step t-1
    feeds the step-t gate matmuls with no decoder-embed roundtrip.
  - GRU algebra refolded so only 5 elementwise ops sit on the chain
    (q -> nin -> tanh -> w -> h'); sigmoid halves, (1-z) and z*h run on
    GpSimd off the critical path.
  - t = tanh(base + u) computed for XB items per group as a single ACT
    activation with per-partition bias (no DVE add), remaining items as one
    batched DVE broadcast-add + one batched ACT tanh.
  - softmax-1 normalisation deferred: U2e = W2SH @ exp(attn1) and the
    1/sum scale is applied after the matmul (one recip + one mult).
  - the argmax tail stays in column form [S, GB] with zero transposes:
    logits+penalty are written straight into the log-prob slab (DVE add),
    GpSimd partition_all_reduce(max) replicates the per-item max across
    partitions, one is_equal gives the one-hot that feeds the next step's
    gate matmuls; ptr indices come from an iota^T @ onehot matmul.
All loop compute is fp32 (f32r only in the prologue; bf16/f32r in the loop
flip tours per the baseline's measurements).
"""
import sys
import numpy as np

for _p in ("/opt/trn_rl_repo",):
    if _p not in sys.path:
        sys.path.insert(0, _p)

B, SS, DS, H, S = 64, 8, 4, 128, 128
NCORES = 8
BL = B // NCORES          # batch items per core = 8
NG = 2                    # pipeline groups per core
GB = BL // NG             # batch items per group = 4
NSTEP = S - 1             # 127
NEG = -1e30
XB1 = 2                   # items using ACT-bias route in stage 1
XB2 = 2                   # items using ACT-bias route in stage 2

# --- packed-constant column layouts ---
_C128 = {}
_off = 0
for _nm, _w in [("I128", H), ("ones128", H), ("WhhT_r", H), ("WhhT_z", H),
                ("WhhT_nh", H), ("w1hT", H), ("vv1c", 1), ("vv2c", 1)]:
    _C128[_nm] = (_off, _off + _w)
    _off += _w
C128_W = _off

_C8 = {}
_off = 0
for _nm, _w in [("staticT8", BL * S), ("dynT4", BL * S), ("A1sT", H),
                ("A1dT", H), ("A2sT", H), ("A2dT", H), ("W2T", H),
                ("MrT", H), ("MzT", H), ("MnT", H)]:
    _C8[_nm] = (_off, _off + _w)
    _off += _w
C8_W = _off

CB_W = BL + 1             # cb: [S, BL+1] = penT columns + iota column


def _build_nc(n_steps=NSTEP, bench_loop=1):
    from contextlib import ExitStack
    import concourse.bass as bass
    import concourse.tile as tile
    from concourse import bacc, mybir, bass_isa

    f32 = mybir.dt.float32
    f32r = mybir.dt.float32r
    AF = mybir.ActivationFunctionType
    OP = mybir.AluOpType

    nc = bacc.Bacc("TRN2", target_bir_lowering=False, debug=False,
                   enable_asserts=False)

    d128 = nc.dram_tensor("c128", [H, C128_W], f32, kind="ExternalInput").ap()
    d8 = nc.dram_tensor("c8", [SS, C8_W], f32, kind="ExternalInput").ap()
    db = nc.dram_tensor("cb", [S, CB_W], f32, kind="ExternalInput").ap()

    nchunk = (GB * n_steps + S - 1) // S
    out_idx = nc.dram_tensor("out_idx_raw", [NG, GB * n_steps], f32,
                             kind="ExternalOutput").ap()
    out_logp = nc.dram_tensor("out_logp_raw", [H, NG * nchunk], f32,
                              kind="ExternalOutput").ap()

    with ExitStack() as ctx:
        tc = ctx.enter_context(tile.TileContext(nc))
        cpool = ctx.enter_context(tc.tile_pool(name="consts", bufs=1))
        state = ctx.enter_context(tc.tile_pool(name="state", bufs=1))
        work = ctx.enter_context(tc.tile_pool(name="work", bufs=3))

        c128 = cpool.tile([H, C128_W], f32, tag="c128")
        c8 = cpool.tile([SS, C8_W], f32, tag="c8")
        cb = cpool.tile([S, CB_W], f32, tag="cb")
        nc.sync.dma_start(c128[:], d128[:])
        nc.sync.dma_start(c8[:], d8[:])
        nc.sync.dma_start(cb[:], db[:])

        def k128(nm):
            lo, hi = _C128[nm]
            return c128[:, lo:hi]

        def k8(nm, rows=SS):
            lo, hi = _C8[nm]
            return c8[0:rows, lo:hi]

        iotac = cb[:, BL:BL + 1]

        # ---- persistent state ----
        base1P = state.tile([H, BL * S], f32, tag="base1P")
        base2P = state.tile([H, BL * S], f32, tag="base2P")
        W2SHT = state.tile([S, BL * H], f32, tag="W2SHT")
        MrTs = state.tile([S, BL * H], f32, tag="MrTs")
        MzTs = state.tile([S, BL * H], f32, tag="MzTs")
        MnTs = state.tile([S, BL * H], f32, tag="MnTs")
        hT = [state.tile([H, GB], f32, tag=f"hT{g}", name=f"hT_{g}")
              for g in range(NG)]
        ohT = [state.tile([S, GB], f32, tag=f"ohT{g}", name=f"ohT_{g}")
               for g in range(NG)]
        mxr = [state.tile([S, GB], f32, tag=f"mxr{g}", name=f"mxr_{g}")
               for g in range(NG)]
        pen = [state.tile([S, GB], f32, tag=f"pen{g}", name=f"pen_{g}")
               for g in range(NG)]
        logbT = [state.tile([S, GB * n_steps], f32, tag=f"logbT{g}",
                            name=f"logbT_{g}") for g in range(NG)]
        ptrS = [state.tile([1, GB * n_steps], f32, tag=f"ptrS{g}",
                           name=f"ptrS_{g}") for g in range(NG)]
        u1sb = [state.tile([H, max(XB1, 1)], f32, tag=f"u1sb{g}",
                           name=f"u1sb_{g}") for g in range(NG)]
        u2sb = [state.tile([H, GB], f32, tag=f"u2sb{g}", name=f"u2sb_{g}")
                for g in range(NG)]
        t1S = [state.tile([H, GB * S], f32, tag=f"t1S{g}", name=f"t1S_{g}")
               for g in range(NG)]
        t2S = [state.tile([H, GB * S], f32, tag=f"t2S{g}", name=f"t2S_{g}")
               for g in range(NG)]
        e1S = [state.tile([S, GB], f32, tag=f"e1S{g}", name=f"e1S_{g}")
               for g in range(NG)]

        for g in range(NG):
            nc.vector.memset(hT[g][:], 0.0)
            nc.vector.memset(ohT[g][:], 0.0)
            nc.vector.memset(logbT[g][:], 0.0)
            nc.vector.tensor_copy(out=pen[g][:],
                                  in_=cb[:, g * GB:(g + 1) * GB])

        # ---- prologue: bases + per-item transposed slabs, no transposes ----
        _cp_flip = [0]

        def copy_ps(dst, src):
            # alternate PSUM->SBUF copies between DVE and ACT
            _cp_flip[0] ^= 1
            if _cp_flip[0]:
                nc.vector.tensor_copy(out=dst, in_=src)
            else:
                nc.scalar.copy(dst, src)

        with tc.tile_pool(name="pro_ps", bufs=2, space="PSUM") as pps:
            def big_mm_to(dst, terms):
                for half in range(2):
                    sl = slice(half * 512, half * 512 + 512)
                    pt = pps.tile([H, 512], f32, tag="pro")
                    for i, (lhsT, rhs) in enumerate(terms):
                        nc.tensor.matmul(pt[:], lhsT, rhs[:, sl],
                                         start=(i == 0),
                                         stop=(i == len(terms) - 1))
                    copy_ps(dst[:, sl], pt[:])

            big_mm_to(base1P, [(k8("A1sT"), k8("staticT8")),
                               (k8("A1dT", DS), k8("dynT4", DS))])
            big_mm_to(base2P, [(k8("A2sT"), k8("staticT8")),
                               (k8("A2dT", DS), k8("dynT4", DS))])

            # per-item [S, H] slabs: (M @ static_b).T = static_b.T @ M.T
            st = _C8["staticT8"][0]
            for dst, mt in [(W2SHT, "W2T"), (MrTs, "MrT"),
                            (MzTs, "MzT"), (MnTs, "MnT")]:
                for b in range(BL):
                    lhsT = c8[:, st + b * S: st + (b + 1) * S]
                    pt = pps.tile([S, H], f32, tag="pro2")
                    nc.tensor.matmul(pt[:], lhsT, k8(mt),
                                     start=True, stop=True)
                    copy_ps(dst[:, b * H:(b + 1) * H], pt[:])

        # ---- main-loop PSUM pools (per group) ----
        psA = [ctx.enter_context(
            tc.tile_pool(name=f"Ag{g}", bufs=1, space="PSUM")) for g in range(NG)]
        psB = [ctx.enter_context(
            tc.tile_pool(name=f"Bg{g}", bufs=1, space="PSUM")) for g in range(NG)]
        bkA = [psA[g].tile([H, 512], f32, tag="bka", name=f"bkA_{g}")
               for g in range(NG)]
        bkB = [psB[g].tile([H, 512], f32, tag="bkb", name=f"bkB_{g}")
               for g in range(NG)]

        AFt, AFe = AF.Tanh, AF.Exp

        def p1(t, g):
            """gates -> GRU h' update."""
            ga = bkA[g]
            G_r, G_z = ga[:, 0:4], ga[:, 4:8]
            G_rz, G_n, G_h2 = ga[:, 0:8], ga[:, 8:12], ga[:, 12:16]
            h_g = hT[g][:]
            oh_g = ohT[g][:]

            # ---- GRU gates (PE) ----
            nc.tensor.matmul(G_r, k128("WhhT_r"), h_g, start=True, stop=False)
            nc.tensor.matmul(G_z, k128("WhhT_z"), h_g, start=True, stop=False)
            nc.tensor.matmul(G_h2, k128("WhhT_nh"), h_g, start=True, stop=True)
            for bl in range(GB):
                b = g * GB + bl
                hs = slice(b * H, (b + 1) * H)
                oc = oh_g[:, bl:bl + 1]
                nc.tensor.matmul(G_r[:, bl:bl + 1], MrTs[:, hs], oc,
                                 start=False, stop=True)
                nc.tensor.matmul(G_z[:, bl:bl + 1], MzTs[:, hs], oc,
                                 start=False, stop=True)
                nc.tensor.matmul(G_n[:, bl:bl + 1], MnTs[:, hs], oc,
                                 start=True, stop=True)
            trz = work.tile([H, 2 * GB], f32, tag=f"trz{g}")
            nc.scalar.activation(trz[:], G_rz, AFt, scale=0.5)
            # on-chain: q -> nin -> tanh -> w -> h'; z2/z2m/a fill tn's gap
            q = work.tile([H, GB], f32, tag=f"q{g}")
            nc.vector.scalar_tensor_tensor(out=q[:], in0=trz[:, 0:GB],
                                           scalar=1.0, in1=G_h2,
                                           op0=OP.add, op1=OP.mult)
            nin = work.tile([H, GB], f32, tag=f"nin{g}")
            nc.vector.tensor_tensor(out=nin[:], in0=q[:], in1=G_n, op=OP.add)
            tn = work.tile([H, GB], f32, tag=f"tn{g}")
            nc.scalar.activation(tn[:], nin[:], AFt)
            z2 = work.tile([H, GB], f32, tag=f"z2{g}")
            nc.vector.tensor_scalar(out=z2[:], in0=trz[:, GB:2 * GB],
                                    scalar1=0.5, scalar2=0.5,
                                    op0=OP.mult, op1=OP.add)
            z2m = work.tile([H, GB], f32, tag=f"z2m{g}")
            nc.vector.tensor_scalar(out=z2m[:], in0=trz[:, GB:2 * GB],
                                    scalar1=-0.5, scalar2=0.5,
                                    op0=OP.mult, op1=OP.add)
            a_ = work.tile([H, GB], f32, tag=f"a{g}")
            nc.vector.tensor_tensor(out=a_[:], in0=z2[:], in1=h_g, op=OP.mult)
            w_ = work.tile([H, GB], f32, tag=f"w{g}")
            nc.vector.tensor_tensor(out=w_[:], in0=z2m[:], in1=tn[:],
                                    op=OP.mult)
            nc.vector.tensor_tensor(out=h_g, in0=w_[:], in1=a_[:], op=OP.add)

        def p2(t, g):
            """stage 1: t1 = tanh(base1 + w1h @ h') -> A1T."""
            ga = bkA[g]
            U1, A1T = ga[:, 16:20], ga[:, 20:24]
            h_g = hT[g][:]
            nc.tensor.matmul(U1, k128("w1hT"), h_g, start=True, stop=True)
            if XB1 > 0:
                nc.vector.tensor_copy(out=u1sb[g][:, 0:XB1], in_=U1[:, 0:XB1])
                for bl in range(XB1):
                    b = g * GB + bl
                    nc.scalar.activation(t1S[g][:, bl * S:(bl + 1) * S],
                                         base1P[:, b * S:(b + 1) * S], AFt,
                                         bias=u1sb[g][:, bl:bl + 1])
            if XB1 < GB:
                nb = GB - XB1
                t1p = work.tile([H, nb * S], f32, tag=f"t1p{g}",
                                name=f"t1p_{g}")[:]
                gsc = slice((g * GB + XB1) * S, (g + 1) * GB * S)
                nc.vector.tensor_tensor(
                    out=t1p.rearrange("p (b s) -> p b s", b=nb),
                    in0=base1P[:, gsc].rearrange("p (b s) -> p b s", b=nb),
                    in1=U1[:, XB1:GB, None].broadcast_to((H, nb, S)),
                    op=OP.add)
                nc.scalar.activation(t1S[g][:, XB1 * S:GB * S], t1p, AFt)
            for bl in range(GB):
                nc.tensor.matmul(A1T[:, bl:bl + 1],
                                 t1S[g][:, bl * S:(bl + 1) * S], k128("vv1c"),
                                 start=True, stop=True)

        def p3(t, g):
            """exp -> U2e -> u2 scale."""
            ga = bkA[g]
            A1T = ga[:, 20:24]
            S1rep, U2e = ga[:, 24:28], ga[:, 28:32]

            nc.scalar.activation(e1S[g][:], A1T, AFe)   # softmax1 numerator
            nc.tensor.matmul(S1rep, k128("ones128"), e1S[g][:],
                             start=True, stop=True)
            for bl in range(GB):
                b = g * GB + bl
                nc.tensor.matmul(U2e[:, bl:bl + 1], W2SHT[:, b * H:(b + 1) * H],
                                 e1S[g][:, bl:bl + 1], start=True, stop=True)
            rH = work.tile([H, GB], f32, tag=f"rH{g}")
            nc.vector.reciprocal(rH[:], S1rep)
            nc.vector.tensor_tensor(out=u2sb[g][:], in0=rH[:], in1=U2e,
                                    op=OP.mult)

        def p4(t, g):
            """stage 2 tanh -> logits -> one-hot -> bookkeeping."""
            ga, gb_ = bkA[g], bkB[g]
            A2T = gb_[:, 0:4]
            P1row = gb_[0:1, 4:8]
            oh_g = ohT[g][:]
            # ---- stage 2: t2 = tanh(base2 + u2), attn2 columns ----
            for bl in range(XB2):
                b = g * GB + bl
                nc.scalar.activation(t2S[g][:, bl * S:(bl + 1) * S],
                                     base2P[:, b * S:(b + 1) * S], AFt,
                                     bias=u2sb[g][:, bl:bl + 1])
            if XB2 < GB:
                nb = GB - XB2
                t2p = work.tile([H, nb * S], f32, tag=f"t2p{g}",
                                name=f"t2p_{g}")[:]
                gsc = slice((g * GB + XB2) * S, (g + 1) * GB * S)
                nc.vector.tensor_tensor(
                    out=t2p.rearrange("p (b s) -> p b s", b=nb),
                    in0=base2P[:, gsc].rearrange("p (b s) -> p b s", b=nb),
                    in1=u2sb[g][:, XB2:GB, None].broadcast_to((H, nb, S)),
                    op=OP.add)
                nc.scalar.activation(t2S[g][:, XB2 * S:GB * S], t2p, AFt)
            for bl in range(GB):
                nc.tensor.matmul(A2T[:, bl:bl + 1],
                                 t2S[g][:, bl * S:(bl + 1) * S], k128("vv2c"),
                                 start=True, stop=True)

            # ---- logits -> slab, replicated max, one-hot (column form) ----
            slab = logbT[g][:, t * GB:(t + 1) * GB]
            nc.vector.tensor_tensor(out=slab, in0=A2T, in1=pen[g][:],
                                    op=OP.add)
            nc.gpsimd.partition_all_reduce(mxr[g][:], slab, S,
                                           bass_isa.ReduceOp.max)
            nc.vector.tensor_tensor(out=oh_g, in0=slab, in1=mxr[g][:],
                                    op=OP.is_equal)
            # off-chain: penalty update (Pool) + ptr via iota dot (PE)
            tsp = work.tile([S, GB], f32, tag=f"tsp{g}")
            nc.gpsimd.tensor_scalar(out=tsp[:], in0=oh_g, scalar1=NEG,
                                    scalar2=None, op0=OP.mult)
            nc.gpsimd.tensor_tensor(out=pen[g][:], in0=pen[g][:], in1=tsp[:],
                                    op=OP.add)
            nc.tensor.matmul(P1row, iotac, oh_g, start=True, stop=True)
            nc.vector.tensor_copy(out=ptrS[g][0:1, t * GB:(t + 1) * GB],
                                  in_=P1row)

        # software-pipelined emission at quarter-step granularity: group 1
        # runs half a step behind group 0, so each engine's in-order stream
        # approximates execution time order.
        for t in range(n_steps):
            p1(t, 0)
            if t > 0:
                p3(t - 1, 1)
            p2(t, 0)
            if t > 0:
                p4(t - 1, 1)
            p3(t, 0)
            p1(t, 1)
            p4(t, 0)
            p2(t, 1)
        p3(n_steps - 1, 1)
        p4(n_steps - 1, 1)

        # ---- post-loop: logp = -ln(sum(exp(logits - max))) ----
        sums = [state.tile([S, nchunk], f32, tag=f"sums{g}",
                           name=f"sums_{g}") for g in range(NG)]
        logpb = [state.tile([S, nchunk], f32, tag=f"logpb{g}",
                            name=f"logpb_{g}") for g in range(NG)]
        for g in range(NG):
            nc.vector.memset(sums[g][:], 1.0)
            for c in range(nchunk):
                w0 = c * S
                wid = min(S, GB * n_steps - w0)
                pt = psB[g].tile([S, S], f32, tag="bkb", name=f"pT{g}{c}")
                nc.tensor.transpose(pt[0:wid, :],
                                    logbT[g][:, w0:w0 + wid], k128("I128"))
                blk = work.tile([S, S], f32, tag=f"pb{g}")
                nc.vector.tensor_copy(out=blk[0:wid, :], in_=pt[0:wid, :])
                nmx = work.tile([S, 1], f32, tag=f"nm{g}")
                nc.vector.tensor_reduce(out=nmx[0:wid, :], in_=blk[0:wid, :],
                                        op=OP.max, axis=mybir.AxisListType.X,
                                        negate=True)
                eb = work.tile([S, S], f32, tag=f"eb{g}")
                nc.scalar.activation(eb[0:wid, :], blk[0:wid, :], AFe,
                                     bias=nmx[0:wid, :],
                                     accum_out=sums[g][0:wid, c:c + 1])
            lnb = work.tile([S, nchunk], f32, tag=f"lnb{g}")
            nc.scalar.activation(lnb[:], sums[g][:], AF.Ln)
            nc.vector.tensor_scalar(out=logpb[g][:], in0=lnb[:], scalar1=-1.0,
                                    scalar2=None, op0=OP.mult)
            nc.sync.dma_start(out_idx[g:g + 1, :], ptrS[g][:])
            nc.sync.dma_start(out_logp[:, g * nchunk:(g + 1) * nchunk],
                              logpb[g][:])

    nc.compile()
    return nc


def host_inputs(static, dynamic, W_s, W_d, W_dec, vv1, ww1, vv2, ww2,
                W_ih, W_hh):
    """Per-core in_maps: weight folds + packing only (data compute on-device)."""
    f = np.float32
    c128 = np.zeros((H, C128_W), f)

    def put128(nm, arr):
        lo, hi = _C128[nm]
        c128[:, lo:hi] = arr
    put128("I128", np.eye(H, dtype=f))
    put128("ones128", np.ones((H, H), f))
    put128("WhhT_r", W_hh[:H].T)
    put128("WhhT_z", W_hh[H:2 * H].T)
    put128("WhhT_nh", 0.5 * W_hh[2 * H:].T)
    put128("w1hT", ww1[:, 2 * H:].T)
    put128("vv1c", vv1[:, None])
    put128("vv2c", vv2[:, None])

    c8s = np.zeros((SS, C8_W), f)

    def put8(nm, arr):
        lo, hi = _C8[nm]
        c8s[:arr.shape[0], lo:lo + arr.shape[1]] = arr
    put8("A1sT", (ww1[:, :H] @ W_s).T)
    put8("A1dT", (ww1[:, H:2 * H] @ W_d).T)
    put8("A2sT", (ww2[:, :H] @ W_s).T)
    put8("A2dT", (ww2[:, 2 * H:] @ W_d).T)
    put8("W2T", (ww2[:, H:2 * H] @ W_s).T)
    put8("MrT", (W_ih[:H] @ W_dec).T)
    put8("MzT", (W_ih[H:2 * H] @ W_dec).T)
    put8("MnT", (W_ih[2 * H:] @ W_dec).T)

    in_maps = []
    for c in range(NCORES):
        bs = slice(c * BL, (c + 1) * BL)
        c8 = c8s.copy()
        lo, _ = _C8["staticT8"]
        c8[:, lo:lo + BL * S] = static[bs].transpose(1, 0, 2).reshape(SS, BL * S)
        lo, _ = _C8["dynT4"]
        c8[:DS, lo:lo + BL * S] = dynamic[bs].transpose(1, 0, 2).reshape(DS, BL * S)
        pen = np.where(dynamic[bs, 0, :] != 0, NEG, 0.0).astype(f)
        pen[:, 0] = NEG
        cbm = np.zeros((S, CB_W), f)
        cbm[:, 0:BL] = pen.T
        cbm[:, BL] = np.arange(S, dtype=f)
        in_maps.append({"c128": c128, "c8": np.ascontiguousarray(c8),
                        "cb": cbm})
    return in_maps


def unpack_outputs(results, n_steps=NSTEP):
    nchunk = (GB * n_steps + S - 1) // S
    idxs, logps = [], []
    for res in results:
        raw_i = res["out_idx_raw"]           # [NG, GB*n_steps]
        idx = np.zeros((BL, n_steps), np.int32)
        for g in range(NG):
            idx[g * GB:(g + 1) * GB, :] = \
                np.rint(raw_i[g].reshape(n_steps, GB).T).astype(np.int32)
        idxs.append(idx)
        raw = res["out_logp_raw"]
        lp = np.zeros((BL, n_steps), np.float32)
        for g in range(NG):
            flat = raw[:, g * nchunk:(g + 1) * nchunk].T.reshape(-1)
            lp[g * GB:(g + 1) * GB, :] = \
                flat[:GB * n_steps].reshape(n_steps, GB).T
        logps.append(lp)
    return np.concatenate(idxs, 0), np.concatenate(logps, 0)


_CACHE = {}


def kernel(static, dynamic, transition_time, W_s, b_s, W_d, b_d, W_dec, b_dec,
           vv1, ww1, vv2, ww2, W_ih, W_hh, b_ih, b_hh):
    for bias in (b_s, b_d, b_dec, b_ih, b_hh):
        assert not np.any(np.asarray(bias)), "kernel assumes zero biases"
    from concourse.bass_utils import run_bass_kernel_spmd
    if "nc" not in _CACHE:
        _CACHE["nc"] = _build_nc()
    in_maps = host_inputs(np.asarray(static), np.asarray(dynamic),
                          np.asarray(W_s), np.asarray(W_d), np.asarray(W_dec),
                          np.asarray(vv1), np.asarray(ww1), np.asarray(vv2),
                          np.asarray(ww2), np.asarray(W_ih), np.asarray(W_hh))
    res = run_bass_kernel_spmd(_CACHE["nc"], in_maps,
                               core_ids=list(range(NCORES)))
    return unpack_outputs(res.results)


# revision 22
# speedup vs baseline: 1.5688x; 1.0198x over previous
"""Trainium2 Bass kernel for nn_DRL4SSP (pointer-network greedy decode).

Strategy: pure data-parallel over batch B=64 across 8 NeuronCores (8 items
per core), two pipeline groups of 4 items interleaved per core. The 127
decode steps are latency-bound on the cross-engine dependency chain, so the
step is built to minimise serial stages:

  - GRU input path folded: M_x = W_ih_x @ W_dec is folded on the host, and
    (M_x @ static_b).T slabs are built in the prologue directly as
    static_b.T @ M_x.T (no transpose pass), so the one-hot from step t-1
    feeds the step-t gate matmuls with no decoder-embed roundtrip.
  - GRU algebra refolded so only 5 elementwise ops sit on the chain
    (q -> nin -> tanh -> w -> h'); sigmoid halves, (1-z) and z*h run on
    GpSimd off the critical path.
  - t = tanh(base + u) computed for XB items per group as a single ACT
    activation with per-partition bias (no DVE add), remaining items as one
    batched DVE broadcast-add + one batched ACT tanh.
  - softmax-1 normalisation deferred: U2e = W2SH @ exp(attn1) and the
    1/sum scale is applied after the matmul (one recip + one mult).
  - the argmax tail stays in column form [S, GB] with zero transposes:
    logits+penalty are written straight into the log-prob slab (DVE add),
    GpSimd partition_all_reduce(max) replicates the per-item max across
    partitions, one is_equal gives the one-hot that feeds the next step's
    gate matmuls; ptr indices come from an iota^T @ onehot matmul.
All loop compute is fp32 (f32r only in the prologue; bf16/f32r in the loop
flip tours per the baseline's measurements).
"""
import sys
import numpy as np

for _p in ("/opt/trn_rl_repo",):
    if _p not in sys.path:
        sys.path.insert(0, _p)

B, SS, DS, H, S = 64, 8, 4, 128, 128
NCORES = 8
BL = B // NCORES          # batch items per core = 8
NG = 2                    # pipeline groups per core
GB = BL // NG             # batch items per group = 4
NSTEP = S - 1             # 127
NEG = -1e30
XB1 = 2                   # items using ACT-bias route in stage 1
XB2 = 2                   # items using ACT-bias route in stage 2

# --- packed-constant column layouts ---
_C128 = {}
_off = 0
for _nm, _w in [("I128", H), ("ones128", H), ("WhhT_r", H), ("WhhT_z", H),
                ("WhhT_nh", H), ("w1hT", H), ("vv1c", 1), ("vv2c", 1)]:
    _C128[_nm] = (_off, _off + _w)
    _off += _w
C128_W = _off

_C8 = {}
_off = 0
for _nm, _w in [("sd12", BL * S), ("A1p", H), ("A2p", H), ("Mp", 4 * H)]:
    _C8[_nm] = (_off, _off + _w)
    _off += _w
C8_W = _off
SD12 = 12                 # packed [static(8); dynamic(4)] contract rows

CB_W = BL + 1             # cb: [S, BL+1] = penT columns + iota column


def _build_nc(n_steps=NSTEP, bench_loop=1):
    from contextlib import ExitStack
    import concourse.bass as bass
    import concourse.tile as tile
    from concourse import bacc, mybir, bass_isa

    f32 = mybir.dt.float32
    f32r = mybir.dt.float32r
    AF = mybir.ActivationFunctionType
    OP = mybir.AluOpType

    nc = bacc.Bacc("TRN2", target_bir_lowering=False, debug=False,
                   enable_asserts=False)

    d128 = nc.dram_tensor("c128", [H, C128_W], f32, kind="ExternalInput").ap()
    d8 = nc.dram_tensor("c8", [SD12, C8_W], f32, kind="ExternalInput").ap()
    db = nc.dram_tensor("cb", [S, CB_W], f32, kind="ExternalInput").ap()

    nchunk = (GB * n_steps + S - 1) // S
    out_idx = nc.dram_tensor("out_idx_raw", [NG, GB * n_steps], f32,
                             kind="ExternalOutput").ap()
    out_logp = nc.dram_tensor("out_logp_raw", [H, NG * nchunk], f32,
                              kind="ExternalOutput").ap()

    with ExitStack() as ctx:
        tc = ctx.enter_context(tile.TileContext(nc))
        cpool = ctx.enter_context(tc.tile_pool(name="consts", bufs=1))
        state = ctx.enter_context(tc.tile_pool(name="state", bufs=1))
        work = ctx.enter_context(tc.tile_pool(name="work", bufs=3))

        c128 = cpool.tile([H, C128_W], f32, tag="c128")
        c8 = cpool.tile([SD12, C8_W], f32, tag="c8")
        cb = cpool.tile([S, CB_W], f32, tag="cb")
        nc.sync.dma_start(c128[:], d128[:])
        nc.sync.dma_start(c8[:], d8[:])
        nc.sync.dma_start(cb[:], db[:])

        def k128(nm):
            lo, hi = _C128[nm]
            return c128[:, lo:hi]

        def k8(nm, rows=SD12):
            lo, hi = _C8[nm]
            return c8[0:rows, lo:hi]

        iotac = cb[:, BL:BL + 1]

        # ---- persistent state ----
        base1P = state.tile([H, BL * S], f32, tag="base1P")
        base2P = state.tile([H, BL * S], f32, tag="base2P")
        SLAB = state.tile([S, BL * 4 * H], f32, tag="SLAB")

        def slab_view(b, j):
            # j: 0=W2SH, 1=Mr, 2=Mz, 3=Mn -> [S, H] block of item b
            lo = b * 4 * H + j * H
            return SLAB[:, lo:lo + H]
        hT = [state.tile([H, GB], f32, tag=f"hT{g}", name=f"hT_{g}")
              for g in range(NG)]
        ohT = [state.tile([S, GB], f32, tag=f"ohT{g}", name=f"ohT_{g}")
               for g in range(NG)]
        mxr = [state.tile([S, GB], f32, tag=f"mxr{g}", name=f"mxr_{g}")
               for g in range(NG)]
        pen = [state.tile([S, GB], f32, tag=f"pen{g}", name=f"pen_{g}")
               for g in range(NG)]
        logbT = [state.tile([S, GB * n_steps], f32, tag=f"logbT{g}",
                            name=f"logbT_{g}") for g in range(NG)]
        ptrS = [state.tile([1, GB * n_steps], f32, tag=f"ptrS{g}",
                           name=f"ptrS_{g}") for g in range(NG)]
        u1sb = [state.tile([H, max(XB1, 1)], f32, tag=f"u1sb{g}",
                           name=f"u1sb_{g}") for g in range(NG)]
        u2sb = [state.tile([H, GB], f32, tag=f"u2sb{g}", name=f"u2sb_{g}")
                for g in range(NG)]
        t1S = [state.tile([H, GB * S], f32, tag=f"t1S{g}", name=f"t1S_{g}")
               for g in range(NG)]
        t2S = [state.tile([H, GB * S], f32, tag=f"t2S{g}", name=f"t2S_{g}")
               for g in range(NG)]
        e1S = [state.tile([S, GB], f32, tag=f"e1S{g}", name=f"e1S_{g}")
               for g in range(NG)]

        for g in range(NG):
            nc.vector.memset(hT[g][:], 0.0)
            nc.vector.memset(ohT[g][:], 0.0)
            nc.vector.memset(logbT[g][:], 0.0)
            nc.vector.tensor_copy(out=pen[g][:],
                                  in_=cb[:, g * GB:(g + 1) * GB])

        # ---- prologue: bases + per-item transposed slabs, no transposes ----
        _cp_flip = [0]

        def copy_ps(dst, src):
            # alternate PSUM->SBUF copies between DVE and ACT
            _cp_flip[0] ^= 1
            if _cp_flip[0]:
                nc.vector.tensor_copy(out=dst, in_=src)
            else:
                nc.scalar.copy(dst, src)

        pps = ctx.enter_context(tc.tile_pool(name="pro_ps", bufs=2,
                                             space="PSUM"))

        def big_mm_to(dst, lhsT):
            for half in range(2):
                sl = slice(half * 512, half * 512 + 512)
                pt = pps.tile([H, 512], f32, tag="pro")
                nc.tensor.matmul(pt[:], lhsT, k8("sd12")[:, sl],
                                 start=True, stop=True)
                copy_ps(dst[:, sl], pt[:])

        def slab_build(b):
            # item b: all four [S, H] blocks in one matmul:
            # ([W2|Mr|Mz|Mn] @ static_b).T = static_b.T @ Mpack
            st = _C8["sd12"][0]
            lhsT = c8[0:SS, st + b * S: st + (b + 1) * S]
            pt = pps.tile([S, 4 * H], f32, tag="pro2")
            nc.tensor.matmul(pt[:], lhsT, k8("Mp", SS), start=True, stop=True)
            copy_ps(SLAB[:, b * 4 * H:(b + 1) * 4 * H], pt[:])

        big_mm_to(base1P, k8("A1p"))
        big_mm_to(base2P, k8("A2p"))

        # ---- main-loop PSUM pools (per group) ----
        psA = [ctx.enter_context(
            tc.tile_pool(name=f"Ag{g}", bufs=1, space="PSUM")) for g in range(NG)]
        psB = [ctx.enter_context(
            tc.tile_pool(name=f"Bg{g}", bufs=1, space="PSUM")) for g in range(NG)]
        bkA = [psA[g].tile([H, 512], f32, tag="bka", name=f"bkA_{g}")
               for g in range(NG)]
        bkB = [psB[g].tile([H, 512], f32, tag="bkb", name=f"bkB_{g}")
               for g in range(NG)]

        AFt, AFe = AF.Tanh, AF.Exp

        def p1(t, g):
            """gates -> GRU h' update."""
            ga = bkA[g]
            G_r, G_z = ga[:, 0:4], ga[:, 4:8]
            G_rz, G_n, G_h2 = ga[:, 0:8], ga[:, 8:12], ga[:, 12:16]
            h_g = hT[g][:]
            oh_g = ohT[g][:]

            # ---- GRU gates (PE) ----
            nc.tensor.matmul(G_r, k128("WhhT_r"), h_g, start=True, stop=False)
            nc.tensor.matmul(G_z, k128("WhhT_z"), h_g, start=True, stop=False)
            nc.tensor.matmul(G_h2, k128("WhhT_nh"), h_g, start=True, stop=True)
            for bl in range(GB):
                b = g * GB + bl
                oc = oh_g[:, bl:bl + 1]
                nc.tensor.matmul(G_r[:, bl:bl + 1], slab_view(b, 1), oc,
                                 start=False, stop=True)
                nc.tensor.matmul(G_z[:, bl:bl + 1], slab_view(b, 2), oc,
                                 start=False, stop=True)
                nc.tensor.matmul(G_n[:, bl:bl + 1], slab_view(b, 3), oc,
                                 start=True, stop=True)
            trz = work.tile([H, 2 * GB], f32, tag=f"trz{g}")
            nc.scalar.activation(trz[:], G_rz, AFt, scale=0.5)
            # on-chain: q -> nin -> tanh -> w -> h'; z2/z2m/a fill tn's gap
            q = work.tile([H, GB], f32, tag=f"q{g}")
            nc.vector.scalar_tensor_tensor(out=q[:], in0=trz[:, 0:GB],
                                           scalar=1.0, in1=G_h2,
                                           op0=OP.add, op1=OP.mult)
            nin = work.tile([H, GB], f32, tag=f"nin{g}")
            nc.vector.tensor_tensor(out=nin[:], in0=q[:], in1=G_n, op=OP.add)
            tn = work.tile([H, GB], f32, tag=f"tn{g}")
            nc.scalar.activation(tn[:], nin[:], AFt)
            z2 = work.tile([H, GB], f32, tag=f"z2{g}")
            nc.vector.tensor_scalar(out=z2[:], in0=trz[:, GB:2 * GB],
                                    scalar1=0.5, scalar2=0.5,
                                    op0=OP.mult, op1=OP.add)
            z2m = work.tile([H, GB], f32, tag=f"z2m{g}")
            nc.vector.tensor_scalar(out=z2m[:], in0=trz[:, GB:2 * GB],
                                    scalar1=-0.5, scalar2=0.5,
                                    op0=OP.mult, op1=OP.add)
            a_ = work.tile([H, GB], f32, tag=f"a{g}")
            nc.vector.tensor_tensor(out=a_[:], in0=z2[:], in1=h_g, op=OP.mult)
            w_ = work.tile([H, GB], f32, tag=f"w{g}")
            nc.vector.tensor_tensor(out=w_[:], in0=z2m[:], in1=tn[:],
                                    op=OP.mult)
            nc.vector.tensor_tensor(out=h_g, in0=w_[:], in1=a_[:], op=OP.add)

        def p2(t, g):
            """stage 1: t1 = tanh(base1 + w1h @ h') -> A1T."""
            ga = bkA[g]
            U1, A1T = ga[:, 16:20], ga[:, 20:24]
            h_g = hT[g][:]
            nc.tensor.matmul(U1, k128("w1hT"), h_g, start=True, stop=True)
            if XB1 > 0:
                nc.vector.tensor_copy(out=u1sb[g][:, 0:XB1], in_=U1[:, 0:XB1])
                for bl in range(XB1):
                    b = g * GB + bl
                    nc.scalar.activation(t1S[g][:, bl * S:(bl + 1) * S],
                                         base1P[:, b * S:(b + 1) * S], AFt,
                                         bias=u1sb[g][:, bl:bl + 1])
            if XB1 < GB:
                nb = GB - XB1
                t1p = work.tile([H, nb * S], f32, tag=f"t1p{g}",
                                name=f"t1p_{g}")[:]
                gsc = slice((g * GB + XB1) * S, (g + 1) * GB * S)
                nc.vector.tensor_tensor(
                    out=t1p.rearrange("p (b s) -> p b s", b=nb),
                    in0=base1P[:, gsc].rearrange("p (b s) -> p b s", b=nb),
                    in1=U1[:, XB1:GB, None].broadcast_to((H, nb, S)),
                    op=OP.add)
                nc.scalar.activation(t1S[g][:, XB1 * S:GB * S], t1p, AFt)
            for bl in range(GB):
                nc.tensor.matmul(A1T[:, bl:bl + 1],
                                 t1S[g][:, bl * S:(bl + 1) * S], k128("vv1c"),
                                 start=True, stop=True)

        def p3(t, g):
            """exp -> U2e -> u2 scale."""
            ga = bkA[g]
            A1T = ga[:, 20:24]
            S1rep, U2e = ga[:, 24:28], ga[:, 28:32]

            nc.scalar.activation(e1S[g][:], A1T, AFe)   # softmax1 numerator
            nc.tensor.matmul(S1rep, k128("ones128"), e1S[g][:],
                             start=True, stop=True)
            for bl in range(GB):
                b = g * GB + bl
                nc.tensor.matmul(U2e[:, bl:bl + 1], slab_view(b, 0),
                                 e1S[g][:, bl:bl + 1], start=True, stop=True)
            rH = work.tile([H, GB], f32, tag=f"rH{g}")
            nc.vector.reciprocal(rH[:], S1rep)
            nc.vector.tensor_tensor(out=u2sb[g][:], in0=rH[:], in1=U2e,
                                    op=OP.mult)

        def p4(t, g):
            """stage 2 tanh -> logits -> one-hot -> bookkeeping."""
            ga, gb_ = bkA[g], bkB[g]
            A2T = gb_[:, 0:4]
            P1row = gb_[0:1, 4:8]
            oh_g = ohT[g][:]
            # ---- stage 2: t2 = tanh(base2 + u2), attn2 columns ----
            for bl in range(XB2):
                b = g * GB + bl
                nc.scalar.activation(t2S[g][:, bl * S:(bl + 1) * S],
                                     base2P[:, b * S:(b + 1) * S], AFt,
                                     bias=u2sb[g][:, bl:bl + 1])
            if XB2 < GB:
                nb = GB - XB2
                t2p = work.tile([H, nb * S], f32, tag=f"t2p{g}",
                                name=f"t2p_{g}")[:]
                gsc = slice((g * GB + XB2) * S, (g + 1) * GB * S)
                nc.vector.tensor_tensor(
                    out=t2p.rearrange("p (b s) -> p b s", b=nb),
                    in0=base2P[:, gsc].rearrange("p (b s) -> p b s", b=nb),
                    in1=u2sb[g][:, XB2:GB, None].broadcast_to((H, nb, S)),
                    op=OP.add)
                nc.scalar.activation(t2S[g][:, XB2 * S:GB * S], t2p, AFt)
            for bl in range(GB):
                nc.tensor.matmul(A2T[:, bl:bl + 1],
                                 t2S[g][:, bl * S:(bl + 1) * S], k128("vv2c"),
                                 start=True, stop=True)

            # ---- logits -> slab, replicated max, one-hot (column form) ----
            slab = logbT[g][:, t * GB:(t + 1) * GB]
            nc.vector.tensor_tensor(out=slab, in0=A2T, in1=pen[g][:],
                                    op=OP.add)
            nc.gpsimd.partition_all_reduce(mxr[g][:], slab, S,
                                           bass_isa.ReduceOp.max)
            nc.vector.tensor_tensor(out=oh_g, in0=slab, in1=mxr[g][:],
                                    op=OP.is_equal)
            # off-chain: penalty update (Pool) + ptr via iota dot (PE)
            tsp = work.tile([S, GB], f32, tag=f"tsp{g}")
            nc.gpsimd.tensor_scalar(out=tsp[:], in0=oh_g, scalar1=NEG,
                                    scalar2=None, op0=OP.mult)
            nc.gpsimd.tensor_tensor(out=pen[g][:], in0=pen[g][:], in1=tsp[:],
                                    op=OP.add)
            nc.tensor.matmul(P1row, iotac, oh_g, start=True, stop=True)
            nc.vector.tensor_copy(out=ptrS[g][0:1, t * GB:(t + 1) * GB],
                                  in_=P1row)

        sums = [state.tile([S, nchunk], f32, tag=f"sums{g}",
                           name=f"sums_{g}") for g in range(NG)]
        for g in range(NG):
            nc.vector.memset(sums[g][:], 1.0)
        _post_done = [set(), set()]

        def post_chunk(g, c):
            _post_done[g].add(c)
            w0 = c * S
            wid = min(S, GB * n_steps - w0)
            pt = bkB[g][0:wid, 128:256]
            nc.tensor.transpose(pt, logbT[g][:, w0:w0 + wid], k128("I128"))
            blk = work.tile([S, S], f32, tag=f"pb{g}")
            nc.vector.tensor_copy(out=blk[0:wid, :], in_=pt)
            nmx = work.tile([S, 1], f32, tag=f"nm{g}")
            nc.vector.tensor_reduce(out=nmx[0:wid, :], in_=blk[0:wid, :],
                                    op=OP.max, axis=mybir.AxisListType.X,
                                    negate=True)
            eb = work.tile([S, S], f32, tag=f"eb{g}")
            nc.scalar.activation(eb[0:wid, :], blk[0:wid, :], AFe,
                                 bias=nmx[0:wid, :],
                                 accum_out=sums[g][0:wid, c:c + 1])

        def p2z(g):
            """step-0 stage 1: h=0 so t1 = tanh(base1) with no bias."""
            ga = bkA[g]
            A1T = ga[:, 20:24]
            gsc = slice(g * GB * S, (g + 1) * GB * S)
            nc.scalar.activation(t1S[g][:], base1P[:, gsc], AFt)
            for bl in range(GB):
                nc.tensor.matmul(A1T[:, bl:bl + 1],
                                 t1S[g][:, bl * S:(bl + 1) * S], k128("vv1c"),
                                 start=True, stop=True)

        # software-pipelined emission at quarter-step granularity: group 1
        # runs half a step behind group 0, so each engine's in-order stream
        # approximates execution time order. Step 0 runs its h=0 shortcut
        # while the W2SH/M slabs are still building.
        p2z(0)
        for b in range(GB):
            slab_build(b)
        p3(0, 0)
        p2z(1)
        for b in range(GB, BL):
            slab_build(b)
        p4(0, 0)
        for t in range(1, n_steps):
            p1(t, 0)
            p3(t - 1, 1)
            p2(t, 0)
            p4(t - 1, 1)
            p3(t, 0)
            p1(t, 1)
            p4(t, 0)
            p2(t, 1)
            if t % 32 == 0 and t // 32 - 1 >= 0:
                post_chunk(0, t // 32 - 1)
                post_chunk(1, t // 32 - 1)
        p3(n_steps - 1, 1)
        p4(n_steps - 1, 1)

        # ---- post: logp = -ln(sum(exp(logits - max))); chunks overlap loop
        for g in range(NG):
            for c in [c for c in range(nchunk) if c not in _post_done[g]]:
                post_chunk(g, c)
        logpb = [state.tile([S, nchunk], f32, tag=f"logpb{g}",
                            name=f"logpb_{g}") for g in range(NG)]
        for g in range(NG):
            lnb = work.tile([S, nchunk], f32, tag=f"lnb{g}")
            nc.scalar.activation(lnb[:], sums[g][:], AF.Ln)
            nc.vector.tensor_scalar(out=logpb[g][:], in0=lnb[:], scalar1=-1.0,
                                    scalar2=None, op0=OP.mult)
            nc.sync.dma_start(out_idx[g:g + 1, :], ptrS[g][:])
            nc.sync.dma_start(out_logp[:, g * nchunk:(g + 1) * nchunk],
                              logpb[g][:])

    nc.compile()
    return nc


def host_inputs(static, dynamic, W_s, W_d, W_dec, vv1, ww1, vv2, ww2,
                W_ih, W_hh):
    """Per-core in_maps: weight folds + packing only (data compute on-device)."""
    f = np.float32
    c128 = np.zeros((H, C128_W), f)

    def put128(nm, arr):
        lo, hi = _C128[nm]
        c128[:, lo:hi] = arr
    put128("I128", np.eye(H, dtype=f))
    put128("ones128", np.ones((H, H), f))
    put128("WhhT_r", W_hh[:H].T)
    put128("WhhT_z", W_hh[H:2 * H].T)
    put128("WhhT_nh", 0.5 * W_hh[2 * H:].T)
    put128("w1hT", ww1[:, 2 * H:].T)
    put128("vv1c", vv1[:, None])
    put128("vv2c", vv2[:, None])

    c8s = np.zeros((SD12, C8_W), f)

    def put8(nm, arr, row0=0):
        lo, hi = _C8[nm]
        c8s[row0:row0 + arr.shape[0], lo:lo + arr.shape[1]] = arr
    put8("A1p", (ww1[:, :H] @ W_s).T)
    put8("A1p", (ww1[:, H:2 * H] @ W_d).T, SS)
    put8("A2p", (ww2[:, :H] @ W_s).T)
    put8("A2p", (ww2[:, 2 * H:] @ W_d).T, SS)
    mp = np.concatenate([(ww2[:, H:2 * H] @ W_s).T, (W_ih[:H] @ W_dec).T,
                         (W_ih[H:2 * H] @ W_dec).T,
                         (W_ih[2 * H:] @ W_dec).T], axis=1)
    put8("Mp", mp)

    in_maps = []
    for c in range(NCORES):
        bs = slice(c * BL, (c + 1) * BL)
        c8 = c8s.copy()
        lo, _ = _C8["sd12"]
        c8[0:SS, lo:lo + BL * S] = \
            static[bs].transpose(1, 0, 2).reshape(SS, BL * S)
        c8[SS:SD12, lo:lo + BL * S] = \
            dynamic[bs].transpose(1, 0, 2).reshape(DS, BL * S)
        pen = np.where(dynamic[bs, 0, :] != 0, NEG, 0.0).astype(f)
        pen[:, 0] = NEG
        cbm = np.zeros((S, CB_W), f)
        cbm[:, 0:BL] = pen.T
        cbm[:, BL] = np.arange(S, dtype=f)
        in_maps.append({"c128": c128, "c8": np.ascontiguousarray(c8),
                        "cb": cbm})
    return in_maps


def unpack_outputs(results, n_steps=NSTEP):
    nchunk = (GB * n_steps + S - 1) // S
    idxs, logps = [], []
    for res in results:
        raw_i = res["out_idx_raw"]           # [NG, GB*n_steps]
        idx = np.zeros((BL, n_steps), np.int32)
        for g in range(NG):
            idx[g * GB:(g + 1) * GB, :] = \
                np.rint(raw_i[g].reshape(n_steps, GB).T).astype(np.int32)
        idxs.append(idx)
        raw = res["out_logp_raw"]
        lp = np.zeros((BL, n_steps), np.float32)
        for g in range(NG):
            flat = raw[:, g * nchunk:(g + 1) * nchunk].T.reshape(-1)
            lp[g * GB:(g + 1) * GB, :] = \
                flat[:GB * n_steps].reshape(n_steps, GB).T
        logps.append(lp)
    return np.concatenate(idxs, 0), np.concatenate(logps, 0)


_CACHE = {}


def kernel(static, dynamic, transition_time, W_s, b_s, W_d, b_d, W_dec, b_dec,
           vv1, ww1, vv2, ww2, W_ih, W_hh, b_ih, b_hh):
    for bias in (b_s, b_d, b_dec, b_ih, b_hh):
        assert not np.any(np.asarray(bias)), "kernel assumes zero biases"
    from concourse.bass_utils import run_bass_kernel_spmd
    if "nc" not in _CACHE:
        _CACHE["nc"] = _build_nc()
    in_maps = host_inputs(np.asarray(static), np.asarray(dynamic),
                          np.asarray(W_s), np.asarray(W_d), np.asarray(W_dec),
                          np.asarray(vv1), np.asarray(ww1), np.asarray(vv2),
                          np.asarray(ww2), np.asarray(W_ih), np.asarray(W_hh))
    res = run_bass_kernel_spmd(_CACHE["nc"], in_maps,
                               core_ids=list(range(NCORES)))
    return unpack_outputs(res.results)
